# revision 24
# baseline (speedup 1.0000x reference)
"""Trainium2 Bass kernel: 4-layer transformer encoder (B=8,T=512,E=1024,H=16,FF=4096).

Sharding: data-parallel over batch — one sequence per NeuronCore, 8 cores,
no collectives. Activations live feature-major ([128 part, sub, T]) so every
linear layer is a natural PE matmul with no on-device transposes.

v2 highlights over the original feature-major design:
  - The gaussian-bias renormalization is folded ENTIRELY into the scores
    matmul: with a = key-256, b = query-256 (exact in bf16),
      ln gb = -(a-b)^2/2sig^2 = ab/16384 - a^2/32768 - b^2/32768.
    The per-query term -b^2/32768 multiplies numerator and denominator of
    the renormalized attention equally, so it is DROPPED (renorm
    invariance). The other two terms ride along as two extra contraction
    rows appended below each head's 64 features in q/k (rows 64:66):
      k row 64 = a/128,            q row 64 = b/128    (product = ab/16384)
      k row 65 = -a^2/32768 - 2,   q row 65 = 1        (the -2 rescales t2,
    also renorm-invariant, keeping exp outputs comfortably inside fp8e4m3
    range). exp(scores) then IS the biased weight — no separate gb multiply,
    no gb blob. The +1e-5 denominator term is dropped (rel err ~3e-5).
  - t2 = exp(scores) is written by the ACT engine directly as fp8e4m3, and
    V is built as fp8, so the attention-value matmul runs in fp8 DoubleRow
    perf mode (2 k-tiles per instruction).
  - LayerNorm stats run as fp8 DoubleRow ones-matmuls on fp8 copies of z
    (cast on ACT), and the fp32 affine output is applied on ACT
    (Identity, per-partition scale/bias APs) — DVE keeps only the two
    tensor-tensor ops with the token-broadcast stats.
  - 1/x and 1/sqrt(x) are computed as exp(-ln(x)) / exp(-0.5 ln(x)) so the
    whole kernel uses a single ACT table set.

Weights are pre-transposed/tiled/cast to bf16 on the host into exact SBUF
layouts and packed into TWO DRAM blobs (one bf16, one fp32 — the fp32 blob
also carries the per-core input x0). Positional encoding is folded into the
input on the host (it is a function of shapes only).
"""

import math
import sys

import numpy as np

if "/opt/trn_rl_repo" not in sys.path:
    sys.path.insert(0, "/opt/trn_rl_repo")

import ml_dtypes

B, T, E, H, L, FF = 8, 512, 1024, 16, 4, 4096
D = E // H  # 64
P = 128
ES = E // P  # 8 e-subtiles
FS = FF // P  # 32 f-subtiles
KT = T // P  # 4 k/q tiles
LN_EPS = 1e-5
BF16 = ml_dtypes.bfloat16
F8 = ml_dtypes.float8_e4m3

import os as _os

# fp8 sites (validated in numerics sim before enabling)
F8_TV = _os.environ.get("K_F8_TV", "1") == "1"    # t2+v fp8, AV DoubleRow
F8_STATS = _os.environ.get("K_F8_STATS", "1") == "1"  # LN stats fp8 DR
USE_RECIP = _os.environ.get("K_RECIP", "1") == "1"    # DVE approx recip

_CACHE = {}
_MARKS = []


def _layout(n_layers=L):
    """Column layout of the two input blobs. Single source of truth for
    _build_bass (device) and _host_prep (host)."""
    bf, f32, f8 = {}, {}, {}

    def seg(d, name, ncols):
        off = d.get("_total", 0)
        d[name] = (off, ncols)
        d["_total"] = off + ncols

    seg(f8, "qax", H * T)   # rows 0:5: q aux rows (bhi, bhi, blo, 1, 1)
    seg(f8, "kax", H * T)   # rows 0:5: k aux rows (ahi, alo, ahi, kchi, kclo)
    for l in range(n_layers):
        seg(f8, f"wq{l}", ES * ES * P)
        seg(f8, f"wk{l}", ES * ES * P)
        seg(bf, f"wv{l}", 2 * ES * T)
        seg(bf, f"wo{l}", ES * ES * P)
        seg(bf, f"w1{l}", FS * ES * P)
        seg(bf, f"w2{l}", ES * 2 * (FS // 2) * P)
        seg(bf, f"vb{l}", E)
    seg(f32, "x0", ES * T)
    for l in range(n_layers):
        for nm, nc_ in ((f"qb{l}", ES), (f"kb{l}", ES), (f"ob{l}", ES),
                        (f"f1b{l}", FS), (f"f2b{l}", ES), (f"g1{l}", ES),
                        (f"b1{l}", ES), (f"g2{l}", ES), (f"b2{l}", ES)):
            seg(f32, nm, nc_)
    seg(f32, "hw", ES)
    seg(f32, "hb", 1)
    return bf, f32, f8


def _build_bass(n_layers=L, repeats=1):
    import concourse.bass as bass  # noqa: F401
    import concourse.mybir as mybir
    import concourse.tile as tile
    from concourse import bacc
    from concourse.bass import ds, ts
    from contextlib import ExitStack

    fp32 = mybir.dt.float32
    bf16 = mybir.dt.bfloat16
    f8 = mybir.dt.float8e4
    AF = mybir.ActivationFunctionType
    OP = mybir.AluOpType
    PM = mybir.MatmulPerfMode

    nc = bacc.Bacc("TRN2")
    _MARKS.clear()

    def mark(nm):
        _MARKS.append((nm, int(nc.next_id())))

    # ---- DRAM blobs ---------------------------------------------------
    LBF, LF32, LF8 = _layout(n_layers)
    CBF, CF32, CF8 = LBF["_total"], LF32["_total"], LF8["_total"]
    wbf = nc.dram_tensor("wbf", [P, CBF], bf16, kind="ExternalInput")
    wf32 = nc.dram_tensor("wf32", [P, CF32], fp32, kind="ExternalInput")
    w8 = nc.dram_tensor("w8", [P, CF8], f8, kind="ExternalInput")

    def bfw(name, ncols=None, sub=0):
        off, n = LBF[name]
        if ncols is None:
            ncols = n
        return wbf.ap()[:, ds(off + sub, ncols)]

    def f8w(name, ncols=None, sub=0):
        off, n = LF8[name]
        if ncols is None:
            ncols = n
        return w8.ap()[:, ds(off + sub, ncols)]

    y_out = nc.dram_tensor("y", [1, 1], fp32, kind="ExternalOutput")

    # const region of the fp32 blob (everything after x0)
    c0 = LF32["x0"][1]
    NCONST = CF32 - c0

    with tile.TileContext(nc) as tc, ExitStack() as ctx:
        # ---- pools ----------------------------------------------------
        singles = ctx.enter_context(tc.tile_pool(name="singles", bufs=1))
        xpool = ctx.enter_context(tc.tile_pool(name="xpool", bufs=2))
        xbfp = ctx.enter_context(tc.tile_pool(name="xbfp", bufs=1))
        qkp = ctx.enter_context(tc.tile_pool(name="qkp", bufs=1))
        ocat_p = ctx.enter_context(tc.tile_pool(name="ocat", bufs=1))
        h1p = ctx.enter_context(tc.tile_pool(name="h1p", bufs=1))
        z8p = ctx.enter_context(tc.tile_pool(name="z8p", bufs=1))
        wstream = ctx.enter_context(tc.tile_pool(name="wstream", bufs=2))
        small = ctx.enter_context(tc.tile_pool(name="small", bufs=2))
        et_p = ctx.enter_context(tc.tile_pool(name="et", bufs=4))
        rows = ctx.enter_context(tc.tile_pool(name="rows", bufs=4))
        bc_p = ctx.enter_context(tc.tile_pool(name="bc", bufs=2))
        psum = ctx.enter_context(tc.tile_pool(name="psum", bufs=1, space="PSUM"))

        def ps_tile(shape, tag, bufs):
            return psum.tile(shape, fp32, tag=tag, bufs=bufs, name=tag)

        _dmaq = [0]

        def wdma(out, in_):
            _dmaq[0] += 1
            nc.sync.dma_start(out=out, in_=in_)

        # ---- constants ------------------------------------------------
        from concourse.hw_specs import get_activation_tables
        _tables = get_activation_tables(nc.m.arch)
        _set_idx = list(_tables).index("natural_log_exp_and_others")
        _ld = mybir.InstLoadActFuncSet(
            name=nc.get_next_instruction_name(), ins=[], outs=[],
            act_func_set_id=_set_idx)
        nc.scalar.add_instruction(_ld)

        ones_bf = singles.tile([P, 1], bf16)
        nc.vector.memset(ones_bf, 1.0)
        st_dt = f8 if F8_STATS else bf16
        # M=65 (duplicated ones columns), mirroring the AV DoubleRow shape:
        # the dual-fp8 ISA checks reject smaller stationaries (M=1/2/8 fail
        # ldweights, out-partition 32 fails dst check; 65 is known-good).
        # Rows 1..64 of the stat psums are unused copies.
        # padded to stride 80 (dual-fp8 ldweights wants a 16-byte-multiple
        # k-pair stride; cols 65:80 unused)
        ones_st2 = singles.tile([P, 2, 80], st_dt)
        nc.vector.memset(ones_st2, 1.0)
        av_dt = f8 if F8_TV else bf16
        v_aug = singles.tile([P, KT, H, D + 1], av_dt)  # v + ones column
        nc.vector.memset(v_aug[:, :, :, D : D + 1], 1.0)

        # q/k (fp8) with five aux contraction rows below the 64 head
        # features: rows 0:64 features of head h (in subtile h), rows 64:69
        # carry the folded gaussian bias in two-term fp8 splits.
        q_ext = qkp.tile([P, H, T], f8, tag="q")
        k_ext = qkp.tile([P, H, T], f8, tag="k")
        NAUX = 5
        nc.sync.dma_start(
            out=q_ext[D : D + NAUX, :, :],
            in_=f8w("qax")[0:NAUX, :].rearrange("p (h t) -> p h t", h=H))
        nc.sync.dma_start(
            out=k_ext[D : D + NAUX, :, :],
            in_=f8w("kax")[0:NAUX, :].rearrange("p (h t) -> p h t", h=H))

        # one DMA for every small fp32 constant (biases/ln params/head)
        consts = singles.tile([P, NCONST], fp32)
        nc.sync.dma_start(out=consts, in_=wf32.ap()[:, ds(c0, NCONST)])

        def cref(name, col):  # [P, 1] column of a packed const
            off = LF32[name][0] - c0
            return consts[:, ds(off + col, 1)]

        hw_sb = singles.tile([P, ES], fp32)
        nc.sync.dma_start(out=hw_sb, in_=wf32.ap()[:, ds(LF32["hw"][0], ES)])
        hb_sb = singles.tile([1, 1], fp32)
        nc.sync.dma_start(out=hb_sb, in_=wf32.ap()[0:1, ds(LF32["hb"][0], 1)])
        eps_row = singles.tile([1, 1], fp32)
        nc.vector.memset(eps_row, LN_EPS)

        # ---- repeats replay the FULL kernel (input load → layers → head)
        for _rep in range(repeats):
          # ---- input --------------------------------------------------
          x_fm = xpool.tile([P, ES, T], fp32, tag="x")
          nc.sync.dma_start(
            out=x_fm,
            in_=wf32.ap()[:, ds(0, ES * T)].rearrange("p (s t) -> p s t", s=ES))
          x_bf = xbfp.tile([P, ES, T], bf16, tag="xbf")
          nc.vector.tensor_copy(out=x_bf, in_=x_fm)
          x8 = xbfp.tile([P, ES, T], f8, tag="x8", name="x8")
          nc.vector.tensor_copy(out=x8, in_=x_fm)

          for l in range(n_layers):
            # ===== QKV =====
            vb_t = wstream.tile([P, E], bf16, tag="vb", bufs=2)
            wdma(vb_t, bfw(f"vb{l}"))
            mark(f"qkv{l}")

            def qk_tile(m, wt_name, bias, dst):
                # one m-tile = 128 features = heads 2m (ps rows 0:64) and
                # 2m+1 (rows 64:128); each head's bias-add lands its rows at
                # partitions 0:64 of the head's own subtile column.
                # fp8 weights + fp8 x, contracting subtile pairs in DoubleRow.
                wt = wstream.tile([P, ES, P], f8, tag="wqk8", bufs=4, name="wt")
                wdma(wt, f8w(wt_name, ES * P, m * ES * P).rearrange(
                    "p (s q) -> p s q", s=ES))
                ps = ps_tile([P, T], "ps", 2)
                for j in range(ES // 2):
                    nc.tensor.matmul(ps, lhsT=wt[:, 2 * j : 2 * j + 2, :],
                                     rhs=x8[:, 2 * j : 2 * j + 2, :],
                                     start=(j == 0), stop=(j == ES // 2 - 1),
                                     perf_mode=PM.DoubleRow)
                # weights are stored x64 so fp8 stays in its normal range;
                # descale here while applying the bias
                nc.vector.tensor_scalar(
                    dst[0:D, 2 * m, :], ps[0:D, :],
                    scalar1=1.0 / 64.0, scalar2=cref(bias, m)[0:D, :],
                    op0=OP.mult, op1=OP.add)
                nc.vector.tensor_scalar(
                    dst[0:D, 2 * m + 1, :], ps[D:P, :],
                    scalar1=1.0 / 64.0, scalar2=cref(bias, m)[D:P, :],
                    op0=OP.mult, op1=OP.add)

            wvts = {}

            def v_pre(ch):
                wvt = wstream.tile([P, ES, T], bf16, tag=f"wv{ch}", bufs=1,
                                   name="wvt")
                wdma(wvt, bfw(f"wv{l}", ES * T, ch * ES * T).rearrange(
                    "p (s t) -> p s t", s=ES))
                wvts[ch] = wvt

            def v_group(ch, tt):
                wvt = wvts[ch]
                ps = ps_tile([P, T], "ps", 2)
                for s in range(ES):
                    nc.tensor.matmul(ps, lhsT=x_bf[:, s, ts(tt, P)],
                                     rhs=wvt[:, s, :],
                                     start=(s == 0), stop=(s == ES - 1))
                dst = v_aug[:, tt, ch * 8 : (ch + 1) * 8, 0:D]
                nc.vector.tensor_add(
                    out=dst, in0=ps,
                    in1=vb_t[:, ds(ch * T, T)].rearrange(
                        "p (h d) -> p h d", d=D))

            def head_scores(h):
                # scores (incl. folded gaussian bias rows) + exp -> t2 fp8
                t2s = []
                for pair in range(2):
                    ps2 = ps_tile([P, 2, T], "ps2", 2)
                    for i in range(2):
                        kt = pair * 2 + i
                        nc.tensor.matmul(
                            ps2[:, i, :],
                            lhsT=k_ext[0 : D + NAUX, h, ts(kt, P)],
                            rhs=q_ext[0 : D + NAUX, h, :], start=True,
                            stop=True)
                    t2 = et_p.tile([P, 2, T], av_dt, tag="t", bufs=8, name="t2")
                    nc.scalar.activation(t2, ps2, AF.Exp)
                    t2s.append(t2)
                return t2s

            def head_av(h, t2s):
                # attention-value matmuls + renormalization for head h
                pb = (h % 2) * D
                sub = h // 2
                ps_o = ps_tile([D + 1, T], "pso", 2)
                if F8_TV:
                    for j in range(2):
                        nc.tensor.matmul(ps_o, lhsT=v_aug[:, 2 * j : 2 * j + 2, h, :],
                                         rhs=t2s[j], start=(j == 0), stop=(j == 1),
                                         perf_mode=PM.DoubleRow)
                else:
                    for kt in range(KT):
                        nc.tensor.matmul(ps_o, lhsT=v_aug[:, kt, h, :],
                                         rhs=t2s[kt // 2][:, kt % 2, :],
                                         start=(kt == 0), stop=(kt == KT - 1))
                # r = 1/st on DVE; the approx-fast recip's bitwise seed
                # cannot read PSUM, so bounce st through SBUF first
                r_row = rows.tile([1, T], fp32, tag="row", name="rr")
                if USE_RECIP:
                    st_sb = rows.tile([1, T], fp32, tag="row", name="st")
                    nc.vector.tensor_copy(out=st_sb, in_=ps_o[D : D + 1, :])
                    nc.vector.reciprocal_approx_fast(r_row, st_sb)
                else:
                    ln_t = rows.tile([1, T], fp32, tag="row", name="lnr")
                    nc.scalar.activation(ln_t, ps_o[D : D + 1, :], AF.Ln)
                    nc.scalar.activation(r_row, ln_t, AF.Exp, scale=-1.0)
                rb = bc_p.tile([D, T], fp32, tag="rb", bufs=2)
                nc.gpsimd.partition_broadcast(rb, r_row)
                nc.vector.tensor_mul(out=o_cat[pb : pb + D, sub, :],
                                     in0=ps_o[0:D, :], in1=rb)

            def qk2(m):
                qk_tile(m, f"wq{l}", f"qb{l}", q_ext)
                qk_tile(m, f"wk{l}", f"kb{l}", k_ext)

            # Fine-grained interleave: each head's score matmuls (S) are
            # emitted between PE-dense weight-matmul fillers (q/k m-tiles,
            # v tt-groups) so the ACT engine's exp (~2us/head) always has
            # filler behind it in the PE queue and the tensor engine never
            # drains into its slow post-idle p-state. AV(h) is emitted a few
            # fillers after S(h), once exp(h) has surely retired. Peak live
            # t2 tiles is 8 (et pool bufs=8).
            o_cat = ocat_p.tile([P, ES, T], bf16, tag="ocat")
            t2g = {}

            def S(h):
                t2g[h] = head_scores(h)

            def AV(h):
                head_av(h, t2g.pop(h))

            v_pre(0)
            qk2(0)
            S(0)
            qk2(1)
            S(1)
            mark(f"v{l}")
            v_group(0, 0)
            v_group(0, 1)
            S(2)
            v_group(0, 2)
            v_group(0, 3)
            S(3)
            mark(f"attnA{l}")
            v_pre(1)
            qk2(2)
            AV(0); S(4)
            qk2(3)
            AV(1); S(5)
            qk2(4)
            AV(2); S(6)
            qk2(5)
            AV(3); S(7)
            qk2(6)
            AV(4); S(8)
            qk2(7)
            AV(5); S(9)
            mark(f"attnB{l}")
            v_group(1, 0)
            v_group(1, 1)
            AV(6); S(10)
            v_group(1, 2)
            v_group(1, 3)
            AV(7); S(11)
            AV(8); S(12)
            AV(9); S(13)
            AV(10); S(14)
            AV(11); S(15)
            AV(12); AV(13); AV(14); AV(15)
            # ===== out-proj + residual =====
            mark(f"outproj{l}")
            z1 = xpool.tile([P, ES, T], fp32, tag="x")
            for m in range(ES):
                wt = wstream.tile([P, ES, P], bf16, tag="wqk", bufs=4)
                wdma(wt, bfw(f"wo{l}", ES * P, m * ES * P).rearrange(
                    "p (s q) -> p s q", s=ES))
                ps = ps_tile([P, T], "ps", 2)
                for s in range(ES):
                    nc.tensor.matmul(ps, lhsT=wt[:, s, :], rhs=o_cat[:, s, :],
                                     start=(s == 0), stop=(s == ES - 1))
                po = small.tile([P, T], fp32, tag="po")
                nc.scalar.activation(po, ps, AF.Identity,
                                     bias=cref(f"ob{l}", m))
                nc.vector.tensor_add(out=z1[:, m, :], in0=x_fm[:, m, :],
                                     in1=po)

            def layernorm(z, g_name, b_name, want_bf=True):
                # stats via fp8 casts (on ACT) + DoubleRow ones-matmuls
                z8 = z8p.tile([P, ES, T], st_dt, tag="z8", name="z8")
                zq8 = z8p.tile([P, ES, T], st_dt, tag="zq8", name="zq8")
                ps_s1 = ps_tile([D + 1, T], "ps", 2)
                ps_s2 = ps_tile([D + 1, T], "ps", 2)
                for s in range(ES):
                    # z cast on ACT, z^2 on DVE — the two stat inputs for a
                    # subtile materialize in parallel
                    nc.scalar.activation(z8[:, s, :], z[:, s, :], AF.Copy)
                    nc.vector.tensor_mul(out=zq8[:, s, :], in0=z[:, s, :],
                                         in1=z[:, s, :])
                if F8_STATS:
                    npair = ES // 2
                    for j in range(npair):
                        nc.tensor.matmul(ps_s1, lhsT=ones_st2[:, :, 0 : D + 1],
                                         rhs=z8[:, 2 * j : 2 * j + 2, :],
                                         start=(j == 0), stop=(j == npair - 1),
                                         perf_mode=PM.DoubleRow)
                        nc.tensor.matmul(ps_s2, lhsT=ones_st2[:, :, 0 : D + 1],
                                         rhs=zq8[:, 2 * j : 2 * j + 2, :],
                                         start=(j == 0), stop=(j == npair - 1),
                                         perf_mode=PM.DoubleRow)
                else:
                    for s in range(ES):
                        nc.tensor.matmul(ps_s1[0:1, :], lhsT=ones_bf,
                                         rhs=z8[:, s, :],
                                         start=(s == 0), stop=(s == ES - 1))
                        nc.tensor.matmul(ps_s2[0:1, :], lhsT=ones_bf,
                                         rhs=zq8[:, s, :],
                                         start=False, stop=(s == ES - 1))
                mu = rows.tile([1, T], fp32, tag="row", name="mu")
                nc.vector.tensor_scalar_mul(mu, ps_s1[0:1, :], 1.0 / E)
                m2 = rows.tile([1, T], fp32, tag="row", name="m2")
                nc.vector.tensor_scalar_mul(m2, ps_s2[0:1, :], 1.0 / E)
                musq = rows.tile([1, T], fp32, tag="row", name="musq")
                nc.scalar.activation(musq, mu, AF.Square)
                var = rows.tile([1, T], fp32, tag="row", name="var")
                nc.vector.tensor_sub(out=var, in0=m2, in1=musq)
                lnv = rows.tile([1, T], fp32, tag="row", name="lnv")
                nc.scalar.activation(lnv, var, AF.Ln, bias=eps_row)
                rstd = rows.tile([1, T], fp32, tag="row", name="rstd")
                nc.scalar.activation(rstd, lnv, AF.Exp, scale=-0.5)
                crow = rows.tile([1, T], fp32, tag="row", name="crow")
                nc.vector.tensor_mul(out=crow, in0=mu, in1=rstd)
                ab = bc_p.tile([P, T], fp32, tag="ab", bufs=1)
                nc.gpsimd.partition_broadcast(ab, rstd)
                cb = bc_p.tile([P, T], fp32, tag="cb", bufs=1)
                nc.gpsimd.partition_broadcast(cb, crow)
                out_fm = xpool.tile([P, ES, T], fp32, tag="x")
                out_bf = xbfp.tile([P, ES, T], bf16, tag="xbf", name="out_bf") if want_bf else None
                out_f8 = xbfp.tile([P, ES, T], f8, tag="x8", name="out_f8") if want_bf else None
                for s in range(ES):
                    t1 = small.tile([P, T], fp32, tag="t1")
                    nc.vector.tensor_mul(out=t1, in0=z[:, s, :], in1=ab)
                    t2 = small.tile([P, T], fp32, tag="t2")
                    nc.vector.tensor_sub(out=t2, in0=t1, in1=cb)
                    # bf16 result first (it gates the next layer's matmuls);
                    # the fp32 residual copy runs on ACT, off the DVE path
                    if want_bf:
                        nc.vector.tensor_scalar(
                            out_bf[:, s, :], t2,
                            scalar1=cref(g_name, s), scalar2=cref(b_name, s),
                            op0=OP.mult, op1=OP.add)
                    if want_bf:
                        nc.scalar.activation(
                            out_f8[:, s, :], t2, AF.Identity,
                            scale=cref(g_name, s), bias=cref(b_name, s))
                    nc.scalar.activation(
                        out_fm[:, s, :], t2, AF.Identity,
                        scale=cref(g_name, s), bias=cref(b_name, s))
                return out_fm, out_bf, out_f8

            mark(f"ln1_{l}")
            x_fm, x_bf, x8 = layernorm(z1, f"g1{l}", f"b1{l}")

            # ===== FFN =====
            mark(f"ffn1{l}")
            h1 = h1p.tile([P, FS, T], bf16, tag="h1")
            for f in range(FS):
                wt = wstream.tile([P, ES, P], bf16, tag="wqk", bufs=4)
                wdma(wt, bfw(f"w1{l}", ES * P, f * ES * P).rearrange(
                    "p (s q) -> p s q", s=ES))
                ps = ps_tile([P, T], "ps", 2)
                for s in range(ES):
                    nc.tensor.matmul(ps, lhsT=wt[:, s, :], rhs=x_bf[:, s, :],
                                     start=(s == 0), stop=(s == ES - 1))
                nc.scalar.activation(h1[:, f, :], ps, AF.Relu,
                                     bias=cref(f"f1b{l}", f))
            mark(f"ffn2{l}")
            z2 = xpool.tile([P, ES, T], fp32, tag="x")
            for m in range(ES):
                ps = ps_tile([P, T], "ps", 2)
                for chk in range(2):
                    wt = wstream.tile([P, FS // 2, P], bf16, tag="w2", bufs=3)
                    wdma(wt, bfw(f"w2{l}", (FS // 2) * P,
                                 (m * 2 + chk) * (FS // 2) * P).rearrange(
                        "p (f q) -> p f q", f=FS // 2))
                    for fs in range(FS // 2):
                        nc.tensor.matmul(
                            ps, lhsT=wt[:, fs, :], rhs=h1[:, chk * 16 + fs, :],
                            start=(chk == 0 and fs == 0),
                            stop=(chk == 1 and fs == FS // 2 - 1))
                po = small.tile([P, T], fp32, tag="po")
                nc.scalar.activation(po, ps, AF.Identity,
                                     bias=cref(f"f2b{l}", m))
                nc.vector.tensor_add(out=z2[:, m, :], in0=x_fm[:, m, :],
                                     in1=po)
            mark(f"ln2_{l}")
            _want = l != n_layers - 1
            x_fm, x_bf2, x8_2 = layernorm(z2, f"g2{l}", f"b2{l}", want_bf=_want)
            if _want:
                x_bf = x_bf2
                x8 = x8_2

          mark("head")
          # ---- head: y = x[last] . hw + hb ----------------------------
          xl = small.tile([P, ES, 1], fp32, tag="xl")
          nc.vector.tensor_mul(out=xl, in0=x_fm[:, :, T - 1 : T],
                               in1=hw_sb[:, :, None])
          xl_r = small.tile([P, 1], fp32, tag="xlr")
          nc.vector.reduce_sum(xl_r, xl, axis=mybir.AxisListType.XYZW)
          xl_bf = small.tile([P, 1], bf16, tag="xlbf")
          nc.vector.tensor_copy(out=xl_bf, in_=xl_r)
          ps_y = ps_tile([1, 1], "ps", 2)
          nc.tensor.matmul(ps_y, lhsT=ones_bf, rhs=xl_bf, start=True, stop=True)
          y_sb = small.tile([1, 1], fp32, tag="ysb")
          nc.vector.tensor_add(out=y_sb, in0=ps_y, in1=hb_sb)
          nc.sync.dma_start(out=y_out.ap(), in_=y_sb)

    nc.finalize()
    return nc


def _host_prep(inputs, n_layers=L):
    """Build the per-core input maps: two packed blobs in exact SBUF layouts."""
    f32 = np.float32
    LBF, LF32, LF8 = _layout(n_layers)
    CBF, CF32, CF8 = LBF["_total"], LF32["_total"], LF8["_total"]

    def fm(a2d):  # [rows, cols] -> partition-major [P, rows//P, cols]
        rows, cols = a2d.shape
        return np.ascontiguousarray(
            a2d.reshape(rows // P, P, cols).transpose(1, 0, 2))

    def mtiled(a2d):  # lhsT [K, M] -> [M//P, P, K//P, P] per-m-tile contiguous
        arr = fm(a2d)  # [P, K//P, M]
        ksub = arr.shape[1]
        m_t = arr.shape[2] // P
        return np.ascontiguousarray(
            arr.reshape(P, ksub, m_t, P).transpose(2, 0, 1, 3))

    def col(a1d):  # [rows] -> per-partition [P, rows//P]
        return np.ascontiguousarray(a1d.reshape(-1, P).T)

    src = np.asarray(inputs["src"], f32)
    pos = np.arange(B, dtype=f32)[:, None]
    div = np.exp(np.arange(0, E, 2, dtype=f32) * (-math.log(10000.0) / E))
    pe = np.zeros((B, E), f32)
    pe[:, 0::2] = np.sin(pos * div)
    pe[:, 1::2] = np.cos(pos * div)
    x0 = src + pe[:, None, :]  # [B, T, E]

    wbf = np.zeros((P, CBF), BF16)
    wf32 = np.empty((P, CF32), f32)
    w8 = np.zeros((P, CF8), F8)

    def put_bf(name, arr_pm):  # arr_pm: [P, ncols]
        off, n = LBF[name]
        assert arr_pm.shape == (P, n), (name, arr_pm.shape, n)
        wbf[:, off : off + n] = arr_pm

    def put_f32(name, arr_pm):
        off, n = LF32[name]
        assert arr_pm.shape == (P, n), (name, arr_pm.shape, n)
        wf32[:, off : off + n] = arr_pm

    def put_f8(name, arr_pm):
        off, n = LF8[name]
        assert arr_pm.shape == (P, n), (name, arr_pm.shape, n)
        w8[:, off : off + n] = arr_pm

    # aux rows for the folded gaussian bias, in two-term fp8 splits:
    # a = key-256, b = query-256; products carried are bhi*ahi + bhi*alo +
    # blo*ahi + 1*kchi + 1*kclo  (the blo*alo term is negligible).
    idx = np.arange(T, dtype=f32)
    a_c = idx - 256.0
    ahi = (a_c / 128.0).astype(F8).astype(f32)
    alo = (a_c / 128.0 - ahi).astype(F8).astype(f32)
    kc = -(a_c * a_c) / 32768.0 - 2.0
    kchi = kc.astype(F8).astype(f32)
    kclo = (kc - kchi).astype(F8).astype(f32)
    qax = np.zeros((5, H, T), f32)
    kax = np.zeros((5, H, T), f32)
    qax[0] = ahi[None, :]
    qax[1] = ahi[None, :]
    qax[2] = alo[None, :]
    qax[3] = 1.0
    qax[4] = 1.0
    kax[0] = ahi[None, :]
    kax[1] = alo[None, :]
    kax[2] = ahi[None, :]
    kax[3] = kchi[None, :]
    kax[4] = kclo[None, :]
    off, n = LF8["qax"]
    w8[0:5, off : off + n] = qax.reshape(5, -1).astype(F8)
    off, n = LF8["kax"]
    w8[0:5, off : off + n] = kax.reshape(5, -1).astype(F8)

    ipw = np.asarray(inputs["in_proj_w"], f32)
    ipb = np.asarray(inputs["in_proj_b"], f32)
    out_w = np.asarray(inputs["out_w"], f32)
    out_b = np.asarray(inputs["out_b"], f32)
    ff1_w = np.asarray(inputs["ff1_w"], f32)
    ff1_b = np.asarray(inputs["ff1_b"], f32)
    ff2_w = np.asarray(inputs["ff2_w"], f32)
    ff2_b = np.asarray(inputs["ff2_b"], f32)
    ln1_g = np.asarray(inputs["ln1_g"], f32)
    ln1_b = np.asarray(inputs["ln1_b"], f32)
    ln2_g = np.asarray(inputs["ln2_g"], f32)
    ln2_b = np.asarray(inputs["ln2_b"], f32)

    def tiles_pm(mt):  # [m_t, P, ksub, Pq] -> [P, m_t*ksub*Pq] (m-major cols)
        return np.ascontiguousarray(
            mt.transpose(1, 0, 2, 3)).reshape(P, -1)

    for l in range(n_layers):
        put_f8(f"wq{l}", tiles_pm(mtiled((ipw[l, 0:E] * 8.0).T).astype(F8)))
        put_f8(f"wk{l}", tiles_pm(mtiled(
            (ipw[l, E : 2 * E] * 64.0).T).astype(F8)))
        wv_fm = fm(ipw[l, 2 * E : 3 * E].T)  # [P, ES, E]
        wv_ch = np.ascontiguousarray(
            wv_fm.reshape(P, ES, 2, T).transpose(2, 0, 1, 3)).astype(BF16)
        put_bf(f"wv{l}", wv_ch.transpose(1, 0, 2, 3).reshape(P, -1))
        put_bf(f"wo{l}", tiles_pm(mtiled(out_w[l].T).astype(BF16)))
        put_bf(f"w1{l}", tiles_pm(mtiled(ff1_w[l].T).astype(BF16)))
        w2t = mtiled(ff2_w[l].T)  # [ES, P, FS, P]
        w2c = np.ascontiguousarray(
            w2t.reshape(ES, P, 2, FS // 2, P).transpose(0, 2, 1, 3, 4)).astype(BF16)
        put_bf(f"w2{l}", np.ascontiguousarray(
            w2c.transpose(2, 0, 1, 3, 4)).reshape(P, -1))
        put_bf(f"vb{l}", np.broadcast_to(
            ipb[l, 2 * E : 3 * E].astype(BF16), (P, E)))
        put_f32(f"qb{l}", col(ipb[l, 0:E] / 8.0))
        put_f32(f"kb{l}", col(ipb[l, E : 2 * E]))
        put_f32(f"ob{l}", col(out_b[l]))
        put_f32(f"f1b{l}", col(ff1_b[l]))
        put_f32(f"f2b{l}", col(ff2_b[l]))
        put_f32(f"g1{l}", col(ln1_g[l]))
        put_f32(f"b1{l}", col(ln1_b[l]))
        put_f32(f"g2{l}", col(ln2_g[l]))
        put_f32(f"b2{l}", col(ln2_b[l]))
    put_f32("hw", col(np.asarray(inputs["head_w"], f32)[0]))
    put_f32("hb", np.full((P, 1), np.asarray(inputs["head_b"], f32).ravel()[0],
                          f32))

    in_maps = []
    x0_off = LF32["x0"][0]
    for c in range(B):
        m = dict(wbf=wbf, w8=w8)
        mf = wf32.copy()
        mf[:, x0_off : x0_off + ES * T] = fm(x0[c].T).reshape(P, -1)
        m["wf32"] = mf
        in_maps.append(m)
    return in_maps


def kernel(**inputs):
    from concourse.bass_utils import run_bass_kernel_spmd

    if "nc" not in _CACHE:
        _CACHE["nc"] = _build_bass()
    nc = _CACHE["nc"]
    in_maps = _host_prep(inputs)
    res = run_bass_kernel_spmd(nc, in_maps, core_ids=list(range(B)))
    y = np.stack([res.results[c]["y"].reshape(1) for c in range(B)], axis=0)
    return y.astype(np.float32)
